# revision 1
# baseline (speedup 1.0000x reference)
"""Trainium2 Bass kernel for a non-selective (LTI) SSM.

Reference computation (per batch b, channel d):
    h_l = A @ h_{l-1} + Bvec * u[b, d, l]        (h in R^N, A = diag(a))
    y[b, d, l] = Cvec . h_l

Because the system is linear time-invariant and A is diagonal, the scan
collapses into a causal convolution with taps k_j = sum_i C_i a_i^j B_i.
We compute it with a chunked algorithm (chunk length Q = 128):

    y_intra[c] = TQ   @ u[c]      TQ lower-tri Toeplitz from k[0..Q-1]
    s[c]       = P    @ u[c]      end-of-chunk state from in-chunk inputs
    h[c]       = a^Q * h[c-1] + s[c]     (cheap 16-step scan, diagonal)
    y[c]       = y_intra[c] + W @ h[c-1] W[t, i] = C_i a_i^(t+1)

Everything is matmuls on the PE array except the 16-step carry scan.

Sharding: data-parallel over d_model (512 / 8 cores = 64 channels/core);
each core processes S = 4 batches x 64 channels = 256 sequences.
"""

import sys

sys.path.insert(0, "/opt/trn_rl_repo")

import numpy as np

import concourse.bass as bass
import concourse.mybir as mybir
import concourse.tile as tile
from concourse import bacc
from concourse.bass_utils import run_bass_kernel_spmd

N_CORES = 8
BATCH = 4
D_MODEL = 512
SEQ_LEN = 2048
N_STATE = 64
Q = 128                       # chunk length == partition dim
NCHUNK = SEQ_LEN // Q         # 16
D_PER_CORE = D_MODEL // N_CORES  # 64
S = BATCH * D_PER_CORE        # 256 sequences per core
GRP = 4                       # chunks per input DMA group
F32 = mybir.dt.float32
F32R = mybir.dt.float32r      # single-instruction fp32 matmul (2x fp32 tput)
DEFAULT_MM_DTYPE = F32R
N_WARMUP = 12                 # dummy matmuls to lift the PE HAM clock gate

# packed const columns: [TQt | PT | WT]
C_TQT, C_PT, C_WT = 0, Q, Q + N_STATE
C_TOT = Q + N_STATE + Q       # 320


def build_program(mm_dtype=DEFAULT_MM_DTYPE):
    """Build the per-core Bass program (identical on all 8 cores)."""
    nc = bacc.Bacc(None, target_bir_lowering=False)

    MD = mm_dtype
    u_d = nc.declare_dram_parameter("u", [NCHUNK, Q, S], MD, isOutput=False)
    cs_d = nc.declare_dram_parameter("consts", [Q, C_TOT], MD, isOutput=False)
    aq_d = nc.declare_dram_parameter("aq", [N_STATE, 1], F32, isOutput=False)
    y_d = nc.declare_dram_parameter("y", [NCHUNK, Q, S], F32, isOutput=True)

    with tile.TileContext(nc) as tc:
        with (
            tc.tile_pool(name="warm", bufs=1) as wpool,
            tc.tile_pool(name="consts", bufs=1) as cpool,
            tc.tile_pool(name="upool", bufs=NCHUNK // GRP) as upool,
            tc.tile_pool(name="hpool", bufs=NCHUNK) as hpool,
            tc.tile_pool(name="ypool", bufs=4) as ypool,
            tc.tile_pool(name="ps_warm", bufs=1, space="PSUM") as ps_w,
            tc.tile_pool(name="ps_s", bufs=3, space="PSUM") as ps_s,
            tc.tile_pool(name="ps_y", bufs=4, space="PSUM") as ps_y,
        ):
            # ---- PE warm-up: dummy matmuls on zeroed scratch, no data deps.
            # They run during the initial DMA window and lift the HAM clock
            # gate (1.2 -> 2.4 GHz) before the real matmuls start.
            wsrc = wpool.tile([Q, 512], mybir.dt.bfloat16)
            nc.vector.memset(wsrc[:], 0.0)
            wps = ps_w.tile([Q, 384], F32)
            for i in range(N_WARMUP):
                nc.tensor.matmul(wps[:], wsrc[:, :Q], wsrc[:, :384],
                                 start=True, stop=True)

            # ---- input DMAs (sync queue): consts, then u chunk 0 alone (so
            # compute can start as early as possible), then the rest.
            cs = cpool.tile([Q, C_TOT], MD)
            nc.sync.dma_start(out=cs[:], in_=cs_d[:])
            aq = cpool.tile([N_STATE, 1], F32)
            u_tiles = []
            ug_tiles = []
            for g in range(NCHUNK // GRP):
                ug = upool.tile([Q, GRP, S], MD, name="ug", tag="ug")
                ug_tiles.append(ug)
            nc.sync.dma_start(
                out=ug_tiles[0][:, 0, :], in_=u_d[0].transpose([0, 1])
            )
            nc.sync.dma_start(out=aq[:], in_=aq_d[:])
            nc.sync.dma_start(
                out=ug_tiles[0][:, 1:GRP, :],
                in_=u_d[1:GRP].transpose([1, 0, 2]),
            )
            for g in range(1, NCHUNK // GRP):
                nc.sync.dma_start(
                    out=ug_tiles[g][:],
                    in_=u_d[g * GRP:(g + 1) * GRP].transpose([1, 0, 2]),
                )
            for g in range(NCHUNK // GRP):
                for jj in range(GRP):
                    u_tiles.append(ug_tiles[g][:, jj, :])

            tqt = cs[:, C_TQT:C_TQT + Q]
            pt = cs[:, C_PT:C_PT + N_STATE]
            wt = cs[:N_STATE, C_WT:C_WT + Q]

            h_prev = None
            for c in range(NCHUNK):
                # y_intra first: its PSUM drain overlaps the s matmul below,
                # so the accumulating inter matmul doesn't stall on the bank.
                py = ps_y.tile([Q, S], F32, name="py", tag="py")
                nc.tensor.matmul(
                    py[:], tqt, u_tiles[c], start=True, stop=(c == 0)
                )
                # end-of-chunk state contribution s[c] = P @ u[c]
                ps = ps_s.tile([N_STATE, S], F32, name="ps", tag="ps")
                nc.tensor.matmul(ps[:], pt, u_tiles[c], start=True, stop=True)
                # y[c] += W @ h[c-1]
                if c > 0:
                    nc.tensor.matmul(
                        py[:], wt, h_prev[:], start=False, stop=True
                    )
                # carry scan h[c] = a^Q * h[c-1] + s[c]
                h = hpool.tile([N_STATE, S], MD, name="h", tag="h")
                if c == 0:
                    nc.vector.tensor_copy(out=h[:], in_=ps[:])
                else:
                    nc.vector.scalar_tensor_tensor(
                        out=h[:],
                        in0=h_prev[:],
                        scalar=aq[:],
                        in1=ps[:],
                        op0=mybir.AluOpType.mult,
                        op1=mybir.AluOpType.add,
                    )
                yt = ypool.tile([Q, S], F32, name="yt", tag="yt")
                # PSUM->SBUF eviction on ScalarE; DVE is busy with the scan
                nc.scalar.copy(out=yt[:], in_=py[:])
                nc.sync.dma_start(out=y_d[c], in_=yt[:])
                h_prev = h

    nc.compile()
    return nc


def make_params(A, Bvec, Cvec):
    """Host-side precompute of the filter matrices (float64 -> float32)."""
    a = np.diag(np.asarray(A, np.float64))
    B64 = np.asarray(Bvec, np.float64)
    C64 = np.asarray(Cvec, np.float64)
    j = np.arange(Q)
    k = (a[None, :] ** j[:, None]) @ (C64 * B64)        # taps k[0..Q-1]
    TQt = np.zeros((Q, Q), np.float64)                  # TQt[t, jc] = k[jc-t]
    for t in range(Q):
        TQt[t, t:] = k[: Q - t]
    PT = (a[None, :] ** (Q - 1 - j)[:, None]) * B64[None, :]   # (Q, N)
    WT = C64[:, None] * (a[:, None] ** (j[None, :] + 1))       # (N, Q)
    aq = (a ** Q)[:, None]                                      # (N, 1)
    consts = np.zeros((Q, C_TOT), np.float64)
    consts[:, C_TQT:C_TQT + Q] = TQt
    consts[:, C_PT:C_PT + N_STATE] = PT
    consts[:N_STATE, C_WT:C_WT + Q] = WT
    f32c = lambda x: np.ascontiguousarray(x, np.float32)
    return f32c(consts), f32c(aq)


_prog_cache = {}


def get_program(mm_dtype=DEFAULT_MM_DTYPE):
    key = str(mm_dtype)
    if key not in _prog_cache:
        _prog_cache[key] = build_program(mm_dtype)
    return _prog_cache[key]


def shard_inputs(u, A, Bvec, Cvec):
    """FULL inputs -> per-core in_maps."""
    consts, aq = make_params(A, Bvec, Cvec)
    u = np.asarray(u, np.float32)
    in_maps = []
    for core in range(N_CORES):
        us = u[:, core * D_PER_CORE:(core + 1) * D_PER_CORE, :]  # (B, Dc, L)
        us = us.reshape(S, SEQ_LEN).T                            # (L, S)
        us = np.ascontiguousarray(us).reshape(NCHUNK, Q, S)
        in_maps.append({"u": us, "consts": consts, "aq": aq})
    return in_maps


def unshard_output(results):
    """Per-core y shards -> FULL (B, D, L) output."""
    out = np.empty((BATCH, D_MODEL, SEQ_LEN), np.float32)
    for core in range(N_CORES):
        ys = results[core]["y"].reshape(SEQ_LEN, S).T            # (S, L)
        out[:, core * D_PER_CORE:(core + 1) * D_PER_CORE, :] = ys.reshape(
            BATCH, D_PER_CORE, SEQ_LEN
        )
    return out


def kernel(u, A, Bvec, Cvec, L):
    u = np.asarray(u)
    assert u.shape == (BATCH, D_MODEL, SEQ_LEN), u.shape
    nc = get_program()
    in_maps = shard_inputs(u, A, Bvec, Cvec)
    res = run_bass_kernel_spmd(nc, in_maps, list(range(N_CORES)))
    return unshard_output(res.results)



# revision 3
# speedup vs baseline: 1.5928x; 1.5928x over previous
"""Trainium2 Bass kernel for a non-selective (LTI) SSM.

Reference computation (per batch b, channel d):
    h_l = A @ h_{l-1} + Bvec * u[b, d, l]        (h in R^N, A = diag(a))
    y[b, d, l] = Cvec . h_l

Because the system is LTI with diagonal A, the scan collapses into a causal
convolution with taps k_j = sum_i C_i a_i^j B_i.  The taps decay as a_max^j,
so the convolution is effectively banded: we truncate it at NB*128 taps where
NB is the smallest block count whose dropped tail has relative L2 norm below
TAP_TAIL_TOL (NB=1..2 for typical uniform-spectrum A).

The banded convolution is computed as NB matmuls per chunk of 128 timesteps:

    y[c] = sum_b  T_b @ u[c-b],   T_b[t, k] = w_{128*b + t - k}

with chunks processed two at a time (free dim 512) so every matmul is a full
128x128x512 bf16 PE instruction.  No state, no scan, no recurrence.

Sharding: data-parallel over d_model (512 / 8 cores = 64 channels/core);
each core processes S = 4 batches x 64 channels = 256 sequences, with the
time axis laid out partition-major ([t_within_chunk, chunk, seq]) so every
DMA is fully contiguous per partition.  All HBM I/O is bf16 (the 2e-2
tolerance dwarfs bf16 rounding), halving DMA traffic vs fp32.
"""

import sys

sys.path.insert(0, "/opt/trn_rl_repo")

import numpy as np

import concourse.bass as bass
import concourse.mybir as mybir
import concourse.tile as tile
from concourse import bacc
from concourse.bass_utils import run_bass_kernel_spmd

N_CORES = 8
BATCH = 4
D_MODEL = 512
SEQ_LEN = 2048
N_STATE = 64
Q = 128                       # chunk length == partition dim
NCHUNK = SEQ_LEN // Q         # 16
NPAIR = NCHUNK // 2           # 8 (chunks are processed in pairs, N=512)
D_PER_CORE = D_MODEL // N_CORES  # 64
S = BATCH * D_PER_CORE        # 256 sequences per core
F32 = mybir.dt.float32
BF16 = mybir.dt.bfloat16
BF16_NP = mybir.dt.np(mybir.dt.bfloat16)
N_WARMUP = 10                 # dummy matmuls to lift the PE HAM clock gate
TAP_TAIL_TOL = 5e-3           # truncation budget (tolerance gate is 2e-2)

# input DMA groups / output DMA groups, in pair units
IN_GROUPS = [(0, 1), (1, 4), (4, 8)]
OUT_GROUPS = [(0, 3), (3, 6), (6, 8)]


def build_program(nb):
    """Per-core Bass program (identical on all 8 cores) for nb tap blocks."""
    nc = bacc.Bacc(None, target_bir_lowering=False)

    pad = (nb - 1) * S        # zero columns for the left boundary
    ucols = pad + NCHUNK * S

    u_d = nc.declare_dram_parameter("u", [Q, NCHUNK * S], BF16, isOutput=False)
    cs_d = nc.declare_dram_parameter("consts", [Q, nb * Q], BF16, isOutput=False)
    y_d = nc.declare_dram_parameter("y", [Q, NCHUNK * S], BF16, isOutput=True)

    with tile.TileContext(nc) as tc:
        with (
            tc.tile_pool(name="warm", bufs=1) as wpool,
            tc.tile_pool(name="consts", bufs=1) as cpool,
            tc.tile_pool(name="upool", bufs=1) as upool,
            tc.tile_pool(name="ypool", bufs=1) as ypool,
            tc.tile_pool(name="ps_warm", bufs=1, space="PSUM") as ps_w,
            tc.tile_pool(name="ps_y", bufs=4, space="PSUM") as ps_y,
        ):
            # ---- PE warm-up: dummy matmuls on zeroed scratch, no data deps.
            # They run during the initial DMA window and lift the HAM clock
            # gate (1.2 -> 2.4 GHz) before the real matmuls start.
            wsrc = wpool.tile([Q, 512], BF16)
            nc.vector.memset(wsrc[:], 0.0)
            wps = ps_w.tile([Q, 512], F32)
            for _ in range(N_WARMUP):
                nc.tensor.matmul(wps[:], wsrc[:, :Q], wsrc[:], start=True,
                                 stop=True)

            # ---- weights via the scalar queue (keeps the sync queue free
            # for the bulk u transfers)
            cs = cpool.tile([Q, nb * Q], BF16)
            nc.scalar.dma_start(out=cs[:], in_=cs_d[:])

            # ---- input: single SBUF-resident tile, partition-major layout.
            # Left zero-pad supplies u[c-b] for the first chunks.
            ua = upool.tile([Q, ucols], BF16, name="ua", tag="ua")
            if pad:
                nc.vector.memset(ua[:, 0:pad], 0.0)
            for p0, p1 in IN_GROUPS:
                nc.sync.dma_start(
                    out=ua[:, pad + p0 * 2 * S: pad + p1 * 2 * S],
                    in_=u_d[:, p0 * 2 * S: p1 * 2 * S],
                )

            ysb = ypool.tile([Q, NCHUNK * S], BF16, name="ysb", tag="ysb")

            for j in range(NPAIR):
                py = ps_y.tile([Q, 2 * S], F32, name="py", tag="py")
                for b in range(nb):
                    lo = pad + (2 * j - b) * S
                    nc.tensor.matmul(
                        py[:], cs[:, b * Q:(b + 1) * Q],
                        ua[:, lo: lo + 2 * S],
                        start=(b == 0), stop=(b == nb - 1),
                    )
                # PSUM -> SBUF drain with bf16 cast, split across two
                # engines so neither ACT nor DVE becomes the tail.
                o = 2 * j * S
                nc.scalar.copy(out=ysb[:, o: o + S], in_=py[:, 0: S])
                nc.vector.tensor_copy(out=ysb[:, o + S: o + 2 * S],
                                      in_=py[:, S: 2 * S])

            for p0, p1 in OUT_GROUPS:
                nc.sync.dma_start(
                    out=y_d[:, p0 * 2 * S: p1 * 2 * S],
                    in_=ysb[:, p0 * 2 * S: p1 * 2 * S],
                )

    nc.compile()
    return nc


def make_params(A, Bvec, Cvec):
    """Host-side precompute: taps -> block-Toeplitz weights (lhsT layout)."""
    a = np.diag(np.asarray(A, np.float64))
    g = np.asarray(Bvec, np.float64) * np.asarray(Cvec, np.float64)
    t = np.arange(SEQ_LEN)
    w = (a[None, :] ** t[:, None]) @ g          # taps w[0..L-1]
    tail2 = np.cumsum((w ** 2)[::-1])[::-1]     # tail2[d] = sum_{j>=d} w_j^2
    total2 = tail2[0]
    # Output position t of a chunk sees taps d <= (nb-1)*Q + t, so the
    # dropped energy averaged over positions is mean_t tail2[(nb-1)*Q + t].
    nb = NCHUNK
    for k in range(1, NCHUNK):
        lo = (k - 1) * Q
        err2 = tail2[lo:lo + Q].mean() / max(total2, 1e-30)
        if np.sqrt(err2) <= TAP_TAIL_TOL:
            nb = k
            break
    # consts[k, b*Q + t] = T_b[t, k] = w_{b*Q + t - k}  (lhsT is transposed)
    tt, kk = np.meshgrid(np.arange(Q), np.arange(Q), indexing="ij")
    consts = np.zeros((Q, nb * Q), np.float64)
    for b in range(nb):
        d = b * Q + tt - kk                     # tap index per (t, k)
        m = np.where((d >= 0) & (d < SEQ_LEN), w[np.clip(d, 0, SEQ_LEN - 1)],
                     0.0)
        consts[:, b * Q:(b + 1) * Q] = m.T      # [k, t]
    return consts.astype(BF16_NP), nb


_prog_cache = {}


def get_program(nb):
    if nb not in _prog_cache:
        _prog_cache[nb] = build_program(nb)
    return _prog_cache[nb]


def shard_inputs(u, A, Bvec, Cvec):
    """FULL inputs -> (per-core in_maps, nb)."""
    consts, nb = make_params(A, Bvec, Cvec)
    u = np.asarray(u, np.float32)
    in_maps = []
    for core in range(N_CORES):
        us = u[:, core * D_PER_CORE:(core + 1) * D_PER_CORE, :]  # (B, Dc, L)
        us = us.reshape(S, SEQ_LEN).T.reshape(NCHUNK, Q, S)      # (c, t, s)
        us = np.ascontiguousarray(us.transpose(1, 0, 2))         # (t, c, s)
        in_maps.append({
            "u": us.reshape(Q, NCHUNK * S).astype(BF16_NP),
            "consts": consts,
        })
    return in_maps, nb


def unshard_output(results):
    """Per-core y shards -> FULL (B, D, L) fp32 output."""
    out = np.empty((BATCH, D_MODEL, SEQ_LEN), np.float32)
    for core in range(N_CORES):
        ys = np.asarray(results[core]["y"], np.float32)
        ys = ys.reshape(Q, NCHUNK, S).transpose(1, 0, 2)         # (c, t, s)
        ys = ys.reshape(SEQ_LEN, S).T                            # (S, L)
        out[:, core * D_PER_CORE:(core + 1) * D_PER_CORE, :] = ys.reshape(
            BATCH, D_PER_CORE, SEQ_LEN
        )
    return out


def kernel(u, A, Bvec, Cvec, L):
    u = np.asarray(u)
    assert u.shape == (BATCH, D_MODEL, SEQ_LEN), u.shape
    in_maps, nb = shard_inputs(u, A, Bvec, Cvec)
    nc = get_program(nb)
    res = run_bass_kernel_spmd(nc, in_maps, list(range(N_CORES)))
    return unshard_output(res.results)
